# revision 2
# baseline (speedup 1.0000x reference)
"""Trainium2 Bass kernel for StyleGAN2-style 4x4 blur (upfirdn2d, up=down=1,
pad=(2,1)) on x:[8,128,256,256] fp32.

Math: out[i,j] = sum_{p,q in [-2,1]} K[1-p,1-q] * x[i+p, j+q]  (zero-padded),
with K the 4x4 blur kernel. K is rank-1 (outer product), so the conv is
separable: an H-pass with taps from the column factor and a W-pass with taps
from the row factor.

Mapping to hardware: each 1-D conv is a banded-matrix product. Per (b,c)
image (256x256) we run two PSUM-accumulated matmul groups on TensorE using
float32r (relaxed fp32, full-rate at N>=256):

  MM1:  t1[w, h'] = sum_h x[h, w] * BH[h, h']      (H-conv, output transposed)
  MM2:  y[h', w'] = sum_w t1[w, h'] * BW[w, w']    (W-conv, transposes back)

K (contraction) is capped at 128, so each group is 2 accumulating matmuls
over 128-row halves; the 256-wide bands fold the zero padding at the image
borders. float32r keeps fp32 storage (no cast DMAs) at ~tf32 multiply
precision -- measured rel. error vs the fp32 reference ~2e-4. ScalarE and
VectorE evacuate PSUM->SBUF; HWDGE DMAs on both rings stream HBM.

The kernel is DMA-bound: 33.5 MB in + 33.5 MB out per core. Measured
~208 us/core (~90% of the ~187 us HBM roofline); compute fully overlaps.
DMA pattern choices that mattered (measured by dT/dR slope over a hardware
repeat loop, which cancels the ~5 ms axon launch overhead): partition p
holds row pair (2p, 2p+1) so every DMA line is 2KB contiguous (the band
matrix absorbs the permutation), 2 images per dma_start, in/out alternating
across the sync/scalar HWDGE rings, deep tile pools (12 input / 8 output
buffers) to keep enough DMAs in flight.

Sharding: batch dim (8) -> one NeuronCore each; channels (128) map to
sequential images per core.
"""

import os
import sys

sys.path.insert(0, "/opt/trn_rl_repo")

import numpy as np

# DMA layout: "v2" = row-pair interleave (2KB contiguous lines),
# "v1" = half-split (two 1KB chunks per line)
LAYOUT = os.environ.get("BLUR_LAYOUT", "v2")

B, C, H, W = 8, 128, 256, 256
KH = KW = 4
N_CORES = 8


def _band_256(taps):
    """Band matrix Bd[k, n] = taps[1 + n - k] for 0 <= 1+n-k < 4, else 0.

    t_out[n] = sum_k Bd[k, n] * x_in[k] is the 1-D conv
    out[n] = sum_{p=-2..1} taps_coeff[p] x[n+p] with taps_coeff[p] = taps[1-p]
    and zero padding (2 leading, 1 trailing) folded in by truncation.
    """
    Bd = np.zeros((256, 256), dtype=np.float64)
    for n in range(256):
        for d in range(4):
            k = n + 1 - d
            if 0 <= k < 256:
                Bd[k, n] = taps[d]
    return Bd


def _factor_kernel(k2):
    """Rank-1 factorization k2 = outer(u, v) (k2 is an outer product)."""
    k2 = np.asarray(k2, dtype=np.float64)
    uu, ss, vv = np.linalg.svd(k2)
    assert ss[1] < 1e-5 * max(ss[0], 1e-30), "blur kernel is not rank-1"
    u = uu[:, 0] * np.sqrt(ss[0])
    v = vv[0] * np.sqrt(ss[0])
    # fix sign so that outer(u, v) ~ k2 with u mostly positive
    if u.sum() < 0:
        u, v = -u, -v
    return u, v


def _make_bands(k2, layout=None):
    """Returns (bh_sb, bw_sb) as float32 [128, 512] SBUF layouts.

    bh_sb[p, j*256 + n] = BH[2p + j, n] -- input rows interleaved in pairs so
    every DMA partition line is one 2KB-contiguous DRAM chunk (rows 2p, 2p+1).
    bw_sb[p, wb*256 + n] = BW[wb*128 + p, n] -- plain half split (W stays on
    partitions of the intermediate, untouched by the interleave).
    """
    if layout is None:
        layout = LAYOUT
    u, v = _factor_kernel(k2)
    # coefficient of x[i+p] is u[1-p] -> band entry BH[k, n] = u[1 + n - k]
    BH = _band_256(u)
    BW = _band_256(v)
    bw_sb = (
        BW.reshape(2, 128, 256).transpose(1, 0, 2).reshape(128, 512)
    ).astype(np.float32)
    if layout == "v2":
        # permute BH's output columns even/odd so MM2 can pick h' = 2i + par
        # with a contiguous 128-col block: column (par*128+i) holds h'=2i+par
        perm = np.concatenate([np.arange(0, 256, 2), np.arange(1, 256, 2)])
        BH = BH[:, perm]
        bh_sb = BH.reshape(128, 2, 256).reshape(128, 512).astype(np.float32)
    else:
        bh_sb = (
            BH.reshape(2, 128, 256).transpose(1, 0, 2).reshape(128, 512)
        ).astype(np.float32)
    return bh_sb, bw_sb


_NC_CACHE = {}


def _build_nc(n_images, repeats=1, mode="full", layout=None, gsz=2,
              bufs=(12, 4, 8, 3, 3), alt_rings=True, swdge_in=False,
              tri=False, copysplit=False, burst=0):
    """Builds the per-core Bass module.

    gsz: images per input/output DMA (bigger transfers, fewer instructions)
    bufs: (xt, t1, yt, ps1, ps2) tile-pool buffer counts
    alt_rings: alternate in/out DMAs across both HWDGE rings (sync/scalar)
    """
    if layout is None:
        layout = LAYOUT
    import contextlib

    import concourse.bacc as bacc
    import concourse.mybir as mybir
    from concourse.tile import TileContext

    f32 = mybir.dt.float32
    f32r = mybir.dt.float32r

    nc = bacc.Bacc("TRN2", target_bir_lowering=False)
    x = nc.dram_tensor("x", (n_images, 256, 256), f32r, kind="ExternalInput")
    bh = nc.dram_tensor("bh", (128, 512), f32r, kind="ExternalInput")
    bw = nc.dram_tensor("bw", (128, 512), f32r, kind="ExternalInput")
    y = nc.dram_tensor("y", (n_images, 256, 256), f32, kind="ExternalOutput")

    if layout == "v2":
        # partition p holds rows 2p and 2p+1: 2KB-contiguous DMA lines
        x_v = x.rearrange("(cc c2) (p j) w -> cc p c2 j w", c2=gsz, j=2)
        y_v = y.rearrange("(cc c2) (p j) w -> cc p c2 j w", c2=gsz, j=2)
    else:
        # partition p holds rows p and 128+p: two 1KB chunks per image
        x_v = x.rearrange("(cc c2) (j p) w -> cc p c2 j w", c2=gsz, p=128)
        y_v = y.rearrange("(cc c2) (j p) w -> cc p c2 j w", c2=gsz, p=128)

    xt_b, t1_b, yt_b, ps1_b, ps2_b = bufs
    with TileContext(nc) as tc:
        with (
            tc.tile_pool(name="consts", bufs=1) as cpool,
            tc.tile_pool(name="xt", bufs=xt_b) as xpool,
            tc.tile_pool(name="t1", bufs=t1_b) as tpool,
            tc.tile_pool(name="yt", bufs=yt_b) as ypool,
            tc.tile_pool(name="ps1", bufs=ps1_b, space="PSUM") as ps1pool,
            tc.tile_pool(name="ps2", bufs=ps2_b, space="PSUM") as ps2pool,
        ):
            bh_sb = cpool.tile([128, 512], f32r, tag="bh")
            bw_sb = cpool.tile([128, 512], f32r, tag="bw")
            nc.sync.dma_start(out=bh_sb[:], in_=bh[:])
            nc.sync.dma_start(out=bw_sb[:], in_=bw[:])

            loop_ctx = (
                tc.For_i(0, repeats, 1) if repeats > 1 else contextlib.nullcontext()
            )
            with loop_ctx:
                pending_outs = []
                for cc in range(n_images // gsz):
                    in_eng = nc.sync if (not alt_rings or cc % 2 == 0) else nc.scalar
                    out_eng = nc.scalar if (not alt_rings or cc % 2 == 0) else nc.sync
                    if swdge_in:
                        in_eng = nc.gpsimd
                    if tri:
                        # third DGE path: SWDGE carries half the input stream
                        in_eng = nc.sync if cc % 2 == 0 else nc.gpsimd
                        out_eng = nc.scalar
                    xt = xpool.tile([128, 512 * gsz], f32r)
                    in_eng.dma_start(
                        out=xt[:].rearrange("p (c2 j w) -> p c2 j w", c2=gsz, j=2),
                        in_=x_v[cc],
                    )
                    if mode == "dmaonly":
                        out_eng.dma_start(
                            out=y_v[cc],
                            in_=xt[:]
                            .bitcast(f32)
                            .rearrange("p (c2 j w) -> p c2 j w", c2=gsz, j=2),
                        )
                        continue

                    yt = ypool.tile([128, 512 * gsz], f32)
                    for c2 in range(gsz):
                        xo = c2 * 512
                        # MM1: t1[w, h'] = sum_h x[h, w] * BH[h, h']
                        ps1 = ps1pool.tile([128, 512], f32)
                        for wb in range(2):
                            for j in range(2):
                                lhsT = xt[
                                    :,
                                    xo + j * 256 + wb * 128 : xo
                                    + j * 256
                                    + wb * 128
                                    + 128,
                                ]
                                rhs = bh_sb[:, j * 256 : (j + 1) * 256]
                                nc.tensor.matmul(
                                    ps1[:, wb * 256 : (wb + 1) * 256],
                                    lhsT,
                                    rhs,
                                    start=(j == 0),
                                    stop=(j == 1),
                                )

                        t1 = tpool.tile([128, 512], f32r)
                        if copysplit:
                            nc.scalar.copy(out=t1[:, 0:256], in_=ps1[:, 0:256])
                            nc.vector.tensor_copy(
                                out=t1[:, 256:512], in_=ps1[:, 256:512]
                            )
                        else:
                            nc.scalar.copy(out=t1[:], in_=ps1[:])

                        # MM2: y[h', w'] = sum_w t1[w, h'] * BW[w, w']
                        ps2 = ps2pool.tile([128, 512], f32)
                        for par in range(2):
                            for wb in range(2):
                                lhsT = t1[
                                    :,
                                    wb * 256 + par * 128 : wb * 256 + par * 128 + 128,
                                ]
                                rhs = bw_sb[:, wb * 256 : (wb + 1) * 256]
                                nc.tensor.matmul(
                                    ps2[:, par * 256 : (par + 1) * 256],
                                    lhsT,
                                    rhs,
                                    start=(wb == 0),
                                    stop=(wb == 1),
                                )

                        if copysplit:
                            nc.vector.tensor_copy(
                                out=yt[:, c2 * 512 : c2 * 512 + 256],
                                in_=ps2[:, 0:256],
                            )
                            nc.scalar.copy(
                                out=yt[:, c2 * 512 + 256 : (c2 + 1) * 512],
                                in_=ps2[:, 256:512],
                            )
                        else:
                            nc.vector.tensor_copy(
                                out=yt[:, c2 * 512 : (c2 + 1) * 512], in_=ps2[:]
                            )
                    if burst:
                        pending_outs.append((cc, yt))
                        if len(pending_outs) >= burst:
                            for occ, oyt in pending_outs:
                                nc.scalar.dma_start(
                                    out=y_v[occ],
                                    in_=oyt[:].rearrange(
                                        "p (c2 j w) -> p c2 j w", c2=gsz, j=2
                                    ),
                                )
                            pending_outs = []
                    else:
                        out_eng.dma_start(
                            out=y_v[cc],
                            in_=yt[:].rearrange(
                                "p (c2 j w) -> p c2 j w", c2=gsz, j=2
                            ),
                        )
                for occ, oyt in pending_outs:
                    nc.scalar.dma_start(
                        out=y_v[occ],
                        in_=oyt[:].rearrange("p (c2 j w) -> p c2 j w", c2=gsz, j=2),
                    )

    nc.compile()
    return nc


def _get_nc(n_images, repeats=1, mode="full", layout=None, **kw):
    key = (n_images, repeats, mode, layout or LAYOUT, tuple(sorted(kw.items())))
    if key not in _NC_CACHE:
        _NC_CACHE[key] = _build_nc(n_images, repeats, mode, layout, **kw)
    return _NC_CACHE[key]


def _parse_kwstr(kwstr):
    kw = {}
    for part in kwstr.split(","):
        if not part:
            continue
        k, v = part.split("=")
        try:
            kw[k] = int(v)
        except ValueError:
            kw[k] = v
    return kw


def build_for_perf(spec, x, k2, repeats):
    """perf.py hook: returns (nc, in_maps) for a spec "<variant>:<mode>:<kw>"."""
    variant, mode, kwstr = spec.split(":", 2)
    kw = _parse_kwstr(kwstr)
    if variant in ("v1", "v2"):
        nc = _get_nc(C, repeats=repeats, mode=mode, layout=variant, **kw)
        bh_sb, bw_sb = _make_bands(k2, layout=variant)
        x = np.ascontiguousarray(np.asarray(x), dtype=np.float32)
        in_maps = [{"x": x[b], "bh": bh_sb, "bw": bw_sb} for b in range(B)]
        return nc, in_maps
    raise ValueError(f"unknown variant {variant!r}")


def kernel(x, kernel, _trace=False):
    from concourse import bass_utils

    x = np.ascontiguousarray(np.asarray(x), dtype=np.float32)
    k2 = np.asarray(kernel, dtype=np.float32)
    assert x.shape == (B, C, H, W), x.shape
    assert k2.shape == (KH, KW), k2.shape

    bh_sb, bw_sb = _make_bands(k2)

    nc = _get_nc(C)
    in_maps = [{"x": x[b], "bh": bh_sb, "bw": bw_sb} for b in range(B)]
    res = bass_utils.run_bass_kernel_spmd(
        nc, in_maps, core_ids=list(range(N_CORES)), trace=_trace
    )
    out = np.stack([res.results[b]["y"] for b in range(B)], axis=0)
    if _trace:
        return out, res
    return out



# revision 20
# speedup vs baseline: 1.3918x; 1.3918x over previous
"""Trainium2 Bass kernel for StyleGAN2-style 4x4 blur (upfirdn2d, up=down=1,
pad=(2,1)) on x:[8,128,256,256] fp32.

Math: out[i,j] = sum_{p,q in [-2,1]} K[1-p,1-q] * x[i+p, j+q]  (zero-padded),
with K the 4x4 blur kernel. K is rank-1 (outer product), so the conv is
separable: an H-pass (column-factor taps) then a W-pass (row-factor taps),
each a banded-matrix product on TensorE:

  MM1:  t1[w, h'] = sum_h x[h, w] * BH[h, h']      (H-conv, output transposed)
  MM2:  y[h', w'] = sum_w t1[w, h'] * BW[w, w']    (W-conv, transposes back)

The kernel is HBM-bound, so I/O is fp16 (tolerance is 2e-2; fp16 end-to-end
measures ~6e-4): the host casts x to fp16 and packs it to the SBUF layout
(x_dev[p, c*512 + j*256 + w] = x[c, j*128+p, w], "pk"); the device writes
fp16 y in the same packed layout and the host unpacks/upcasts.  33.5 MB in +
33.5 MB out per core fp32 becomes 16.8 + 16.8 MB; measured DMA-only floor is
~101 us (~332 GB/s/core); the full kernel measures ~104 us vs 207-209 us for
the fp32 baseline (dT/dR hardware-repeat slope, which cancels the ~5-80 ms
axon launch noise).

What mattered (all measured):
- Band trimming (trim=3): the 256-wide band matmul wastes 64x MACs; with
  k=128 row slabs the N-streams trim to [0,130) (T,F), [126,130) j1-fixup
  (F,T), [130,256) (T,T) per m-half -> 12 matmuls/image instead of 8xN=256.
  Full-width (m=128, k=128) weights keep FWL on: k=32/64 boundary matmuls
  (trim=2) measured 2.3x SLOWER (unhidden LDWEIGHTS at every tiny matmul).
- fp16 operand dtype keeps PE at 1 cycle/row even for short N (float32r
  drops to 1/4 rate below N=256, so trimming requires 16-bit).
- Evacuation (the PSUM->SBUF cast copies) can only run on ScalarE and
  VectorE (GPSIMD has no PSUM port); one big [128, 1024] copy per PASS PAIR
  (pair=2, 2-bank PSUM tiles) parity-alternated between the two engines.
- MM2 is emitted `skew` slots behind MM1 so PE never stalls on evac1.
- Host-packed DMA (pk) gives gsz*1KB contiguous lines; raw DMA rate is the
  same as 512B descriptors (hard ~332 GB/s/core cap) but full-kernel time
  improves ~7 us (less DGE/semaphore pressure alongside compute).

Sharding: batch dim (8) -> one NeuronCore each; channels (128) map to
sequential images per core.
"""

import os
import sys

sys.path.insert(0, "/opt/trn_rl_repo")

import numpy as np

# DMA layout: "v2" = row-pair interleave (2KB contiguous lines),
# "v1" = half-split (two 1KB chunks per line)
LAYOUT = os.environ.get("BLUR_LAYOUT", "v2")

B, C, H, W = 8, 128, 256, 256
KH = KW = 4
N_CORES = 8


def _band_256(taps):
    """Band matrix Bd[k, n] = taps[1 + n - k] for 0 <= 1+n-k < 4, else 0.

    t_out[n] = sum_k Bd[k, n] * x_in[k] is the 1-D conv
    out[n] = sum_{p=-2..1} taps_coeff[p] x[n+p] with taps_coeff[p] = taps[1-p]
    and zero padding (2 leading, 1 trailing) folded in by truncation.
    """
    Bd = np.zeros((256, 256), dtype=np.float64)
    for n in range(256):
        for d in range(4):
            k = n + 1 - d
            if 0 <= k < 256:
                Bd[k, n] = taps[d]
    return Bd


def _factor_kernel(k2):
    """Rank-1 factorization k2 = outer(u, v) (k2 is an outer product)."""
    k2 = np.asarray(k2, dtype=np.float64)
    uu, ss, vv = np.linalg.svd(k2)
    assert ss[1] < 1e-5 * max(ss[0], 1e-30), "blur kernel is not rank-1"
    u = uu[:, 0] * np.sqrt(ss[0])
    v = vv[0] * np.sqrt(ss[0])
    # fix sign so that outer(u, v) ~ k2 with u mostly positive
    if u.sum() < 0:
        u, v = -u, -v
    return u, v


def _make_bands(k2, layout=None):
    """Returns (bh_sb, bw_sb) as float32 [128, 512] SBUF layouts.

    bh_sb[p, j*256 + n] = BH[2p + j, n] -- input rows interleaved in pairs so
    every DMA partition line is one 2KB-contiguous DRAM chunk (rows 2p, 2p+1).
    bw_sb[p, wb*256 + n] = BW[wb*128 + p, n] -- plain half split (W stays on
    partitions of the intermediate, untouched by the interleave).
    """
    if layout is None:
        layout = LAYOUT
    u, v = _factor_kernel(k2)
    # coefficient of x[i+p] is u[1-p] -> band entry BH[k, n] = u[1 + n - k]
    BH = _band_256(u)
    BW = _band_256(v)
    bw_sb = (
        BW.reshape(2, 128, 256).transpose(1, 0, 2).reshape(128, 512)
    ).astype(np.float32)
    if layout == "v2":
        # permute BH's output columns even/odd so MM2 can pick h' = 2i + par
        # with a contiguous 128-col block: column (par*128+i) holds h'=2i+par
        perm = np.concatenate([np.arange(0, 256, 2), np.arange(1, 256, 2)])
        BH = BH[:, perm]
        bh_sb = BH.reshape(128, 2, 256).reshape(128, 512).astype(np.float32)
    else:
        bh_sb = (
            BH.reshape(2, 128, 256).transpose(1, 0, 2).reshape(128, 512)
        ).astype(np.float32)
    return bh_sb, bw_sb


_NC_CACHE = {}


def _build_nc(n_images, repeats=1, mode="full", layout=None, gsz=2,
              bufs=(12, 4, 8, 3, 3), alt_rings=True, swdge_in=False,
              tri=False, copysplit=False, burst=0):
    """Builds the per-core Bass module.

    gsz: images per input/output DMA (bigger transfers, fewer instructions)
    bufs: (xt, t1, yt, ps1, ps2) tile-pool buffer counts
    alt_rings: alternate in/out DMAs across both HWDGE rings (sync/scalar)
    """
    if layout is None:
        layout = LAYOUT
    import contextlib

    import concourse.bacc as bacc
    import concourse.mybir as mybir
    from concourse.tile import TileContext

    f32 = mybir.dt.float32
    f32r = mybir.dt.float32r

    nc = bacc.Bacc("TRN2", target_bir_lowering=False)
    x = nc.dram_tensor("x", (n_images, 256, 256), f32r, kind="ExternalInput")
    bh = nc.dram_tensor("bh", (128, 512), f32r, kind="ExternalInput")
    bw = nc.dram_tensor("bw", (128, 512), f32r, kind="ExternalInput")
    y = nc.dram_tensor("y", (n_images, 256, 256), f32, kind="ExternalOutput")

    if layout == "v2":
        # partition p holds rows 2p and 2p+1: 2KB-contiguous DMA lines
        x_v = x.rearrange("(cc c2) (p j) w -> cc p c2 j w", c2=gsz, j=2)
        y_v = y.rearrange("(cc c2) (p j) w -> cc p c2 j w", c2=gsz, j=2)
    else:
        # partition p holds rows p and 128+p: two 1KB chunks per image
        x_v = x.rearrange("(cc c2) (j p) w -> cc p c2 j w", c2=gsz, p=128)
        y_v = y.rearrange("(cc c2) (j p) w -> cc p c2 j w", c2=gsz, p=128)

    xt_b, t1_b, yt_b, ps1_b, ps2_b = bufs
    with TileContext(nc) as tc:
        with (
            tc.tile_pool(name="consts", bufs=1) as cpool,
            tc.tile_pool(name="xt", bufs=xt_b) as xpool,
            tc.tile_pool(name="t1", bufs=t1_b) as tpool,
            tc.tile_pool(name="yt", bufs=yt_b) as ypool,
            tc.tile_pool(name="ps1", bufs=ps1_b, space="PSUM") as ps1pool,
            tc.tile_pool(name="ps2", bufs=ps2_b, space="PSUM") as ps2pool,
        ):
            bh_sb = cpool.tile([128, 512], f32r, tag="bh")
            bw_sb = cpool.tile([128, 512], f32r, tag="bw")
            nc.sync.dma_start(out=bh_sb[:], in_=bh[:])
            nc.sync.dma_start(out=bw_sb[:], in_=bw[:])

            loop_ctx = (
                tc.For_i(0, repeats, 1) if repeats > 1 else contextlib.nullcontext()
            )
            with loop_ctx:
                pending_outs = []
                for cc in range(n_images // gsz):
                    in_eng = nc.sync if (not alt_rings or cc % 2 == 0) else nc.scalar
                    out_eng = nc.scalar if (not alt_rings or cc % 2 == 0) else nc.sync
                    if swdge_in:
                        in_eng = nc.gpsimd
                    if tri:
                        # third DGE path: SWDGE carries half the input stream
                        in_eng = nc.sync if cc % 2 == 0 else nc.gpsimd
                        out_eng = nc.scalar
                    xt = xpool.tile([128, 512 * gsz], f32r)
                    in_eng.dma_start(
                        out=xt[:].rearrange("p (c2 j w) -> p c2 j w", c2=gsz, j=2),
                        in_=x_v[cc],
                    )
                    if mode == "dmaonly":
                        out_eng.dma_start(
                            out=y_v[cc],
                            in_=xt[:]
                            .bitcast(f32)
                            .rearrange("p (c2 j w) -> p c2 j w", c2=gsz, j=2),
                        )
                        continue

                    yt = ypool.tile([128, 512 * gsz], f32)
                    for c2 in range(gsz):
                        xo = c2 * 512
                        # MM1: t1[w, h'] = sum_h x[h, w] * BH[h, h']
                        ps1 = ps1pool.tile([128, 512], f32)
                        for wb in range(2):
                            for j in range(2):
                                lhsT = xt[
                                    :,
                                    xo + j * 256 + wb * 128 : xo
                                    + j * 256
                                    + wb * 128
                                    + 128,
                                ]
                                rhs = bh_sb[:, j * 256 : (j + 1) * 256]
                                nc.tensor.matmul(
                                    ps1[:, wb * 256 : (wb + 1) * 256],
                                    lhsT,
                                    rhs,
                                    start=(j == 0),
                                    stop=(j == 1),
                                )

                        t1 = tpool.tile([128, 512], f32r)
                        if copysplit:
                            nc.scalar.copy(out=t1[:, 0:256], in_=ps1[:, 0:256])
                            nc.vector.tensor_copy(
                                out=t1[:, 256:512], in_=ps1[:, 256:512]
                            )
                        else:
                            nc.scalar.copy(out=t1[:], in_=ps1[:])

                        # MM2: y[h', w'] = sum_w t1[w, h'] * BW[w, w']
                        ps2 = ps2pool.tile([128, 512], f32)
                        for par in range(2):
                            for wb in range(2):
                                lhsT = t1[
                                    :,
                                    wb * 256 + par * 128 : wb * 256 + par * 128 + 128,
                                ]
                                rhs = bw_sb[:, wb * 256 : (wb + 1) * 256]
                                nc.tensor.matmul(
                                    ps2[:, par * 256 : (par + 1) * 256],
                                    lhsT,
                                    rhs,
                                    start=(wb == 0),
                                    stop=(wb == 1),
                                )

                        if copysplit:
                            nc.vector.tensor_copy(
                                out=yt[:, c2 * 512 : c2 * 512 + 256],
                                in_=ps2[:, 0:256],
                            )
                            nc.scalar.copy(
                                out=yt[:, c2 * 512 + 256 : (c2 + 1) * 512],
                                in_=ps2[:, 256:512],
                            )
                        else:
                            nc.vector.tensor_copy(
                                out=yt[:, c2 * 512 : (c2 + 1) * 512], in_=ps2[:]
                            )
                    if burst:
                        pending_outs.append((cc, yt))
                        if len(pending_outs) >= burst:
                            for occ, oyt in pending_outs:
                                nc.scalar.dma_start(
                                    out=y_v[occ],
                                    in_=oyt[:].rearrange(
                                        "p (c2 j w) -> p c2 j w", c2=gsz, j=2
                                    ),
                                )
                            pending_outs = []
                    else:
                        out_eng.dma_start(
                            out=y_v[cc],
                            in_=yt[:].rearrange(
                                "p (c2 j w) -> p c2 j w", c2=gsz, j=2
                            ),
                        )
                for occ, oyt in pending_outs:
                    nc.scalar.dma_start(
                        out=y_v[occ],
                        in_=oyt[:].rearrange("p (c2 j w) -> p c2 j w", c2=gsz, j=2),
                    )

    nc.compile()
    return nc


def _get_nc(n_images, repeats=1, mode="full", layout=None, **kw):
    key = (n_images, repeats, mode, layout or LAYOUT, tuple(sorted(kw.items())))
    if key not in _NC_CACHE:
        _NC_CACHE[key] = _build_nc(n_images, repeats, mode, layout, **kw)
    return _NC_CACHE[key]


F16 = np.float16


def _make_bands16(k2):
    """v1-layout fp16 band tiles [128, 512]:
    b[p, j*256 + n] = BAND[j*128 + p, n]   (j = row-half of the 256-row band)
    """
    u, v = _factor_kernel(k2)
    BH = _band_256(u)
    BW = _band_256(v)
    bh = np.concatenate([BH[0:128, :], BH[128:256, :]], axis=1).astype(F16)
    bw = np.concatenate([BW[0:128, :], BW[128:256, :]], axis=1).astype(F16)
    return bh, bw


def _build_nc16(n_images, repeats=1, mode="full", gsz=4, trim=2, skew=2,
                bufs=(6, 5, 6), psb=3, inq="sync", outq="swdge", pair=1,
                layout="v1", dup=1):
    """fp16 kernel: x natural [n_images, 256, 256] f16, v1 slabs on SBUF.

    Per image, per pass (H then W), per m-half (128 of 256 outputs):
      trim=0: 2 accumulating matmuls, N=256 each (no band trimming)
      trim=1: A1 [0,126) N=126 (T,T); A2 [126,130) N=4 (T,F); B1 [126,130)
              N=4 (F,T); B2 [130,256) N=126 (T,T)
      trim=2: A [0,127) N=127 (T,T); Ma k=64@64 [127,130) N=3 (T,F);
              Mb k=32@0 [127,130) N=3 (F,T); B [130,256) N=126 (T,T)
      trim=3: A [0,130) N=130 (T,F); B1 [126,130) N=4 (F,T); B2 [130,256)
              N=126 (T,T) -- 12 matmuls/image instead of 16
    layout: "v1" natural DRAM (512B chunks); "v2" row-pair (1KB, dmaonly
    diagnostics only); "pk" host-packed v1 (gsz*1KB contiguous / partition).
    Evacuation: one PSUM->SBUF cast copy per pass (paired across `pair`
    images), parity-alternated between ScalarE and VectorE (the only
    PSUM-capable engines).
    MM2 is emitted `skew` slots behind MM1 so PE never waits on evac1.
    """
    import contextlib

    import concourse.bacc as bacc
    import concourse.mybir as mybir
    from concourse.tile import TileContext

    f32 = mybir.dt.float32
    f16 = mybir.dt.float16

    nc = bacc.Bacc("TRN2", target_bir_lowering=False)
    if layout == "pk":
        x = nc.dram_tensor("x", (128, n_images * 512), f16, kind="ExternalInput")
        y = nc.dram_tensor("y", (128, n_images * 512), f16, kind="ExternalOutput")
    else:
        x = nc.dram_tensor("x", (n_images, 256, 256), f16, kind="ExternalInput")
        y = nc.dram_tensor("y", (n_images, 256, 256), f16, kind="ExternalOutput")
    bh = nc.dram_tensor("bh", (128, 512), f16, kind="ExternalInput")
    bw = nc.dram_tensor("bw", (128, 512), f16, kind="ExternalInput")

    if layout == "v1":
        # v1 slab views: partition p holds rows p (j=0) and 128+p (j=1);
        # 512B DMA chunks, k=128 contraction slabs
        x_v = x.rearrange("(cc c2) (j p) w -> cc p c2 j w", c2=gsz, p=128)
        y_v = y.rearrange("(cc c2) (j p) w -> cc p c2 j w", c2=gsz, p=128)
    elif layout == "v2":
        # row-pair: 1KB DMA chunks (dmaonly diagnostics only)
        x_v = x.rearrange("(cc c2) (p j) w -> cc p c2 j w", c2=gsz, j=2)
        y_v = y.rearrange("(cc c2) (p j) w -> cc p c2 j w", c2=gsz, j=2)
    elif layout == "pk":
        x_v = y_v = None
    else:
        raise ValueError(layout)

    def dma_in_ap(g):
        if layout == "pk":
            return x[:, g * gsz * 512 : (g + 1) * gsz * 512]
        return x_v[g]

    def dma_out_ap(g):
        if layout == "pk":
            return y[:, g * gsz * 512 : (g + 1) * gsz * 512]
        return y_v[g]

    def sb_ap(t):
        if layout == "pk":
            return t[:]
        return t[:].rearrange("p (c2 j w) -> p c2 j w", c2=gsz, j=2)

    n_groups = n_images // gsz
    assert n_images % gsz == 0

    with TileContext(nc) as tc:
        with (
            tc.tile_pool(name="consts", bufs=1) as cpool,
            tc.tile_pool(name="xt", bufs=bufs[0]) as xpool,
            tc.tile_pool(name="t1", bufs=bufs[1]) as tpool,
            tc.tile_pool(name="yt", bufs=bufs[2]) as ypool,
            tc.tile_pool(name="ps1", bufs=psb, space="PSUM") as ps1pool,
            tc.tile_pool(name="ps2", bufs=psb, space="PSUM") as ps2pool,
        ):
            bh_sb = cpool.tile([128, 512], f16, tag="bh")
            bw_sb = cpool.tile([128, 512], f16, tag="bw")
            nc.sync.dma_start(out=bh_sb[:], in_=bh[:])
            nc.sync.dma_start(out=bw_sb[:], in_=bw[:])

            engs = {
                "sync": nc.sync,
                "scalar": nc.scalar,
                "vector": nc.vector,
                "swdge": nc.gpsimd,
            }
            in_eng = engs[inq]
            out_eng = engs[outq]
            evac = [nc.scalar, nc.vector]

            def copy_out(eng, out, in_):
                if eng is nc.scalar:
                    eng.copy(out=out, in_=in_)
                else:
                    eng.tensor_copy(out=out, in_=in_)

            def band_pass(ps, lhs_of, band, half):
                """One conv pass into ps[:, half*256 ... +256].

                lhs_of(j, klo, khi) -> lhsT AP with contraction partitions
                [klo,khi) of row-slab j, free dim = the m-half's 128 outputs.
                """
                base = half * 256
                l0 = lhs_of(0, 0, 128)
                l1 = lhs_of(1, 0, 128)
                if trim == 0:
                    nc.tensor.matmul(ps[:, base : base + 256], l0,
                                     band[:, 0:256], start=True, stop=False)
                    nc.tensor.matmul(ps[:, base : base + 256], l1,
                                     band[:, 256:512], start=False, stop=True)
                elif trim == 1:
                    nc.tensor.matmul(ps[:, base : base + 126], l0,
                                     band[:, 0:126], start=True, stop=True)
                    nc.tensor.matmul(ps[:, base + 126 : base + 130], l0,
                                     band[:, 126:130], start=True, stop=False)
                    nc.tensor.matmul(ps[:, base + 126 : base + 130], l1,
                                     band[:, 382:386], start=False, stop=True)
                    nc.tensor.matmul(ps[:, base + 130 : base + 256], l1,
                                     band[:, 386:512], start=True, stop=True)
                elif trim == 3:
                    nc.tensor.matmul(ps[:, base : base + 130], l0,
                                     band[:, 0:130], start=True, stop=False,
                                     skip_group_check=True)
                    nc.tensor.matmul(ps[:, base + 126 : base + 130], l1,
                                     band[:, 382:386], start=False, stop=True,
                                     skip_group_check=True)
                    nc.tensor.matmul(ps[:, base + 130 : base + 256], l1,
                                     band[:, 386:512], start=True, stop=True)
                else:
                    nc.tensor.matmul(ps[:, base : base + 127], l0,
                                     band[:, 0:127], start=True, stop=True)
                    nc.tensor.matmul(ps[:, base + 127 : base + 130],
                                     lhs_of(0, 64, 128),
                                     band[64:128, 127:130],
                                     start=True, stop=False)
                    nc.tensor.matmul(ps[:, base + 127 : base + 130],
                                     lhs_of(1, 0, 32),
                                     band[0:32, 383:386],
                                     start=False, stop=True)
                    nc.tensor.matmul(ps[:, base + 130 : base + 256], l1,
                                     band[:, 386:512], start=True, stop=True)

            P = pair
            assert gsz % P == 0 and n_images % P == 0
            n_slots = dup * n_images // P
            loop_ctx = (
                tc.For_i(0, repeats, 1) if repeats > 1 else contextlib.nullcontext()
            )
            with loop_ctx:
                xts, t1s, yts = {}, {}, {}
                for s in range(n_slots + skew):
                    if s < n_slots:
                        g, c2 = divmod(s * P % n_images, gsz)
                        if c2 == 0:
                            xt = xpool.tile([128, 512 * gsz], f16)
                            in_eng.dma_start(out=sb_ap(xt), in_=dma_in_ap(g))
                            xts[g] = xt
                            if mode == "dmaonly":
                                out_eng.dma_start(
                                    out=dma_out_ap(g), in_=sb_ap(xt)
                                )
                        if mode == "dmaonly":
                            continue
                        xt = xts[g]

                        # MM1: t1[w, h'] = sum_h x[h, w] * BH[h, h']
                        ps1 = ps1pool.tile([128, 512 * P], f32)
                        for sub in range(P):
                            xo = (c2 + sub) * 512
                            for wb in range(2):
                                band_pass(
                                    ps1,
                                    lambda j, klo, khi: xt[
                                        klo:khi,
                                        xo + j * 256 + wb * 128 : xo
                                        + j * 256
                                        + wb * 128
                                        + 128,
                                    ],
                                    bh_sb,
                                    2 * sub + wb,
                                )
                        t1 = tpool.tile([128, 512 * P], f16)
                        copy_out(evac[s % 2], t1[:], ps1[:])
                        t1s[s] = t1

                    js = s - skew
                    if js < 0 or mode == "dmaonly":
                        continue
                    jg, jc2 = divmod(js * P % n_images, gsz)
                    if jc2 == 0:
                        yt = ypool.tile([128, 512 * gsz], f16, tag="yt")
                        yts[jg] = yt
                    t1 = t1s.pop(js)
                    # MM2: y[h', w'] = sum_w t1[w, h'] * BW[w, w']
                    ps2 = ps2pool.tile([128, 512 * P], f32)
                    for sub in range(P):
                        to = sub * 512
                        for hh in range(2):
                            band_pass(
                                ps2,
                                lambda j, klo, khi: t1[
                                    klo:khi,
                                    to + j * 256 + hh * 128 : to
                                    + j * 256
                                    + hh * 128
                                    + 128,
                                ],
                                bw_sb,
                                2 * sub + hh,
                            )
                    copy_out(
                        evac[(js + 1) % 2],
                        yts[jg][:, jc2 * 512 : jc2 * 512 + 512 * P],
                        ps2[:],
                    )
                    if jc2 + P == gsz:
                        out_eng.dma_start(
                            out=dma_out_ap(jg), in_=sb_ap(yts[jg])
                        )

    nc.compile()
    return nc


def _get_nc16(n_images, repeats=1, mode="full", **kw):
    key = ("f16", n_images, repeats, mode, tuple(sorted(kw.items())))
    if key not in _NC_CACHE:
        _NC_CACHE[key] = _build_nc16(n_images, repeats, mode, **kw)
    return _NC_CACHE[key]


def _parse_kwstr(kwstr):
    kw = {}
    for part in kwstr.split(","):
        if not part:
            continue
        k, v = part.split("=")
        if "." in v:  # dot-separated int tuple, e.g. bufs=8.6.8
            kw[k] = tuple(int(t) for t in v.split("."))
        else:
            try:
                kw[k] = int(v)
            except ValueError:
                kw[k] = v
    return kw


def build_for_perf(spec, x, k2, repeats):
    """perf.py hook: returns (nc, in_maps) for a spec "<variant>:<mode>:<kw>"."""
    variant, mode, kwstr = spec.split(":", 2)
    kw = _parse_kwstr(kwstr)
    if variant in ("v1", "v2"):
        nc = _get_nc(C, repeats=repeats, mode=mode, layout=variant, **kw)
        bh_sb, bw_sb = _make_bands(k2, layout=variant)
        x = np.ascontiguousarray(np.asarray(x), dtype=np.float32)
        in_maps = [{"x": x[b], "bh": bh_sb, "bw": bw_sb} for b in range(B)]
        return nc, in_maps
    if variant == "f16":
        nc = _get_nc16(C, repeats=repeats, mode=mode, **kw)
        bh, bw = _make_bands16(k2)
        if kw.get("layout", "v1") == "pk":
            x16 = _pack16(np.asarray(x))
        else:
            x16 = np.asarray(x).astype(F16)
        in_maps = [{"x": x16[b], "bh": bh, "bw": bw} for b in range(B)]
        return nc, in_maps
    raise ValueError(f"unknown variant {variant!r}")


def _pack16(x):
    """[B, C, 256, 256] fp32 -> [B, 128, C*512] f16 packed v1:
    out[b, p, c*512 + j*256 + w] = x[b, c, j*128 + p, w]"""
    xb = x.reshape(B, C, 2, 128, W).astype(F16)
    return np.ascontiguousarray(xb.transpose(0, 3, 1, 2, 4)).reshape(B, 128, C * 512)


def _unpack16(yp):
    """[128, C*512] f16 -> [C, 256, 256] f32 (single core)"""
    yv = yp.reshape(128, C, 2, W).transpose(1, 2, 0, 3)
    return np.ascontiguousarray(yv).astype(np.float32).reshape(C, H, W)


# default device config ("f16:full:<kw>" spec tail); overridable for experiments
DEFAULT_CFG = os.environ.get("BLUR_CFG", "f16:full:trim=3,layout=pk,pair=2,psb=2")


def kernel(x, kernel, _trace=False):
    from concourse import bass_utils

    x = np.asarray(x)
    k2 = np.asarray(kernel, dtype=np.float32)
    assert x.shape == (B, C, H, W), x.shape
    assert k2.shape == (KH, KW), k2.shape

    variant, _, kwstr = DEFAULT_CFG.split(":", 2)
    nc, in_maps = build_for_perf(DEFAULT_CFG, x, k2, repeats=1)
    res = bass_utils.run_bass_kernel_spmd(
        nc, in_maps, core_ids=list(range(N_CORES)), trace=_trace
    )
    if variant == "f16" and _parse_kwstr(kwstr).get("layout", "v1") == "pk":
        out = np.stack([_unpack16(res.results[b]["y"]) for b in range(B)], axis=0)
    else:
        out = np.stack([res.results[b]["y"] for b in range(B)], axis=0)
        out = np.ascontiguousarray(out).astype(np.float32)
    if _trace:
        return out, res
    return out

